# revision 11
# baseline (speedup 1.0000x reference)
# Trainium2 Bass kernel for nn_EvoformerGraphDecoderV2.
# Data-parallel over batch: 16 batches -> 8 cores x 2 batches (sequential per core).
# E pair tensor resident in SBUF as bf16 [j, i, c]; token-major tiles [j, c] per i.
# Node-track matmuls in float32r; E-track matmuls + triangular einsum in bf16.
# The triangular-mult output t is stored back into tbbuf channel-by-channel
# (channel c of tbbuf is dead once channel-c's einsum matmul has issued).
import numpy as np
import ml_dtypes

import concourse.bacc as bacc
import concourse.tile as tile
from concourse import mybir
from concourse.bass_utils import run_bass_kernel_spmd
from concourse.masks import make_identity

FP = mybir.dt.float32
BF = mybir.dt.bfloat16
FR = mybir.dt.float32r
AF = mybir.ActivationFunctionType
OP = mybir.AluOpType
AX = mybir.AxisListType

B, M, K = 16, 96, 32
N = M + K            # 128
D, C = 256, 128
DH, CH = 512, 256
H = 8
DHEAD = D // H       # 32
L = 4
GED = 256
NL, NF, EL, EF = 32, 16, 8, 8
NB = 2               # batches per core
NCORES = 8
EPS = 1e-5

BF_NP = ml_dtypes.bfloat16


def _np(x):
    return np.asarray(x, dtype=np.float32)


def _prep_weights(params):
    """Fold LN gammas/betas into downstream weights. All biases in this problem
    are zero (asserted); LN betas are zero, so folded biases vanish too."""
    t = {}
    consts = {}

    def fold(g, b, W, bias):
        Wf = _np(g)[:, None] * _np(W)
        bf = _np(b) @ _np(W) + (_np(bias) if bias is not None else 0.0)
        assert np.max(np.abs(bf)) == 0.0, "nonzero folded bias unsupported"
        return Wf.astype(np.float32)

    def zb(bias):
        assert np.max(np.abs(_np(bias))) == 0.0, "nonzero bias unsupported"

    p = params
    t['gmlp_w1'] = _np(p['mlp_graph']['W1']); zb(p['mlp_graph']['b1'])
    t['gmlp_w2'] = _np(p['mlp_graph']['W2']); zb(p['mlp_graph']['b2'])

    for l, lp in enumerate(p['layers']):
        pre = f'l{l}_'
        g, b = lp['ln_x']['g'], lp['ln_x']['b']
        wq = fold(g, b, lp['Wq'], lp['bq'])
        wk_ = fold(g, b, lp['Wk'], lp['bk'])
        wv = fold(g, b, lp['Wv'], lp['bv'])
        t[pre + 'qkv'] = np.concatenate([wq / np.sqrt(DHEAD), wk_, wv], axis=1)
        t[pre + 'wo'] = _np(lp['Wo']); zb(lp['bo'])
        geb = _np(lp['ln_e_bias']['g'])
        zb(lp['ln_e_bias']['b'])
        wb = geb[:, None] * _np(lp['Wbias'])
        t[pre + 'wb'] = wb.astype(BF_NP)
        consts[pre + 'wb_colsum'] = wb.sum(axis=0)
        g, b = lp['ln_x_mlp']['g'], lp['ln_x_mlp']['b']
        t[pre + 'mx1'] = fold(g, b, lp['mlp_x']['W1'], lp['mlp_x']['b1'])
        t[pre + 'mx2'] = _np(lp['mlp_x']['W2']); zb(lp['mlp_x']['b2'])
        g, b = lp['ln_x_op']['g'], lp['ln_x_op']['b']
        wa = fold(g, b, lp['Wopa'], lp['bopa'])
        wbp = fold(g, b, lp['Wopb'], lp['bopb'])
        t[pre + 'opab'] = np.concatenate([wa, wbp], axis=1)
        t[pre + 'wop'] = _np(lp['Wop']); zb(lp['bop'])
        g, b = lp['ln_e_tri']['g'], lp['ln_e_tri']['b']
        wga = fold(g, b, lp['Wga'], lp['bga'])
        wgb = fold(g, b, lp['Wgb'], lp['bgb'])
        wg = fold(g, b, lp['Wg'], lp['bg'])
        wta = fold(g, b, lp['Wta'], lp['bta'])
        wtb = fold(g, b, lp['Wtb'], lp['btb'])
        # col layout: [ga | gb | g | ta | tb]
        t[pre + 'w5'] = np.concatenate([wga, wgb, wg, wta, wtb], axis=1).astype(BF_NP)
        g, b = lp['ln_t']['g'], lp['ln_t']['b']
        t[pre + 'wto'] = fold(g, b, lp['Wto'], lp['bto']).astype(BF_NP)
        g, b = lp['ln_e_mlp']['g'], lp['ln_e_mlp']['b']
        t[pre + 'me1'] = fold(g, b, lp['mlp_e']['W1'], lp['mlp_e']['b1']).astype(BF_NP)
        t[pre + 'me2'] = _np(lp['mlp_e']['W2']).astype(BF_NP); zb(lp['mlp_e']['b2'])

    t['hne_w1'] = _np(p['head_ne']['W1']); zb(p['head_ne']['b1'])
    t['hne_w2'] = _np(p['head_ne']['W2']); zb(p['head_ne']['b2'])
    t['hnf_w1'] = _np(p['head_nf']['W1']); zb(p['head_nf']['b1'])
    t['hnf_w2'] = _np(p['head_nf']['W2']); zb(p['head_nf']['b2'])
    t['hhnl'] = np.concatenate([_np(p['head_h']['W']), _np(p['head_nl']['W']),
                                np.zeros((D, 1), np.float32)], axis=1)
    zb(p['head_h']['b']); zb(p['head_nl']['b'])
    t['haef'] = np.concatenate(
        [_np(p['head_A']['W']), _np(p['head_el']['W']), _np(p['head_ef']['W1'])],
        axis=1).astype(BF_NP)
    zb(p['head_A']['b']); zb(p['head_el']['b']); zb(p['head_ef']['b1'])
    t['hef2'] = _np(p['head_ef']['W2']).astype(BF_NP); zb(p['head_ef']['b2'])

    t['qn'] = _np(p['queries_nodes'])[0]
    t['qe'] = _np(p['queries_edges'])[0].astype(BF_NP)
    return t, consts


def _build(weights, consts):
    import contextlib
    nc = bacc.Bacc("TRN2", target_bir_lowering=False, debug=False,
                   num_devices=NCORES)

    dram = {}
    for name, arr in weights.items():
        dt = BF if arr.dtype == BF_NP else FP
        dram[name] = nc.dram_tensor(name, list(arr.shape), dt,
                                    kind="ExternalInput").ap()
    ge_in = nc.dram_tensor("ge", [NB, K, GED], FP, kind="ExternalInput").ap()

    o_ne = nc.dram_tensor("o_ne", [NB, M, D], FP, kind="ExternalOutput").ap()
    o_hnl = nc.dram_tensor("o_hnl", [NB, M, 1 + NL], FP, kind="ExternalOutput").ap()
    o_nf = nc.dram_tensor("o_nf", [NB, M, NF], FP, kind="ExternalOutput").ap()
    o_A = nc.dram_tensor("o_A", [NB, M, M], FP, kind="ExternalOutput").ap()
    o_el = nc.dram_tensor("o_el", [NB, M, M, EL], BF, kind="ExternalOutput").ap()
    o_ef = nc.dram_tensor("o_ef", [NB, M, M, EF], BF, kind="ExternalOutput").ap()

    with tile.TileContext(nc) as tc:
        ctx = contextlib.ExitStack()
        with ctx:
            cst = ctx.enter_context(tc.tile_pool(name="cst", bufs=1))
            res = ctx.enter_context(tc.tile_pool(name="res", bufs=1))
            big = ctx.enter_context(tc.tile_pool(name="big", bufs=1))
            wpool = ctx.enter_context(tc.tile_pool(name="wp", bufs=1))
            wk = ctx.enter_context(tc.tile_pool(name="wk", bufs=2))
            wk2 = ctx.enter_context(tc.tile_pool(name="wk2", bufs=2))
            wk3 = ctx.enter_context(tc.tile_pool(name="wk3", bufs=3))
            st = ctx.enter_context(tc.tile_pool(name="st", bufs=1))
            wkc = ctx.enter_context(tc.tile_pool(name="wkc", bufs=1))
            ps_t = ctx.enter_context(tc.tile_pool(name="ps_t", bufs=2, space="PSUM"))
            ps_g = ctx.enter_context(tc.tile_pool(name="ps_g", bufs=2, space="PSUM"))
            ps_w = ctx.enter_context(tc.tile_pool(name="ps_w", bufs=4, space="PSUM"))

            idf = cst.tile([128, 128], FP); make_identity(nc, idf[:])
            idb = cst.tile([128, 128], BF); make_identity(nc, idb[:])
            idr = cst.tile([128, 128], FR); nc.vector.tensor_copy(idr[:], idf[:])
            epsT = cst.tile([128, 1], FP); nc.vector.memset(epsT[:], EPS)

            # resident state
            Ebuf = res.tile([128, N, C], BF, tag="Ebuf")
            tabuf = res.tile([128, N, C], BF, tag="tabuf")
            tbbuf = res.tile([128, N, C], BF, tag="tbbuf")   # tb, then t (aliased)
            sgbuf = res.tile([128, N, C], BF, tag="sgbuf")
            Xb = res.tile([N, D], FP, tag="Xb")
            biasb = res.tile([128, H, 128], BF, tag="biasb")

            COLD = {"wide", "oab", "xn", "px", "pxs"}

            def wtile(shape, dt, tag):
                pool = wkc if tag in COLD else wk
                return pool.tile(shape, dt, tag=tag, name=tag)

            def drain(out, in_, k):
                if k % 2 == 1:
                    nc.scalar.copy(out=out, in_=in_)
                else:
                    nc.vector.tensor_copy(out, in_)

            def sb_ts2(out, in0, s1, s2, k):
                eng = [nc.vector, nc.gpsimd][k % 2]
                eng.tensor_scalar(out=out, in0=in0, scalar1=s1, scalar2=s2,
                                  op0=OP.subtract, op1=OP.mult)

            def psum_add_into(dst_ap, psum_ap, k, tag="padd"):
                """dst += psum, via a bf16 bounce (avoids mixed-dtype tt)."""
                tmp = wk.tile(list(psum_ap.shape), BF, tag="gtmp")
                drain(tmp[:], psum_ap, k)
                d2 = dst_ap.rearrange("p a b -> p (a b)")
                eng = [nc.vector, nc.gpsimd][k % 2]
                eng.tensor_tensor(out=d2, in0=tmp[:], in1=d2, op=OP.add)

            def load_w(name, dt, tag, rnd=False):
                arr = weights[name]
                rows, cols = arr.shape
                nr = (rows + 127) // 128
                tl = wpool.tile([128, nr, cols], dt if not rnd else FP,
                                tag="wld" if rnd else ("w_" + tag))
                src = dram[name]
                if nr > 1:
                    src = src.rearrange("(q p) c -> p q c", q=nr)
                    nc.sync.dma_start(out=tl[:], in_=src)
                else:
                    nc.sync.dma_start(out=tl[:, 0, :], in_=src[:, :])
                if rnd:
                    tr = wpool.tile([128, nr, cols], FR, tag="w_" + tag)
                    for q in range(nr):
                        nc.vector.tensor_copy(tr[:, q, :], tl[:, q, :])
                    return tr
                return tl

            def ln_stats(buf, T, kb=0):
                S1 = st.tile([128, 128], FP, tag="S1")
                nc.vector.tensor_reduce(out=S1[:, 0:T], in_=buf[:, 0:T, :],
                                        axis=AX.X, op=OP.add)
                S2 = st.tile([128, 128], FP, tag="S2")
                for i in range(T):
                    scr = wk.tile([128, C], BF, tag="lnscr")
                    eng = [nc.vector, nc.scalar][(i + kb) % 2]
                    if eng is nc.scalar:
                        nc.scalar.activation(out=scr[:], in_=buf[:, i, :],
                                             func=AF.Square,
                                             accum_out=S2[:, i:i + 1])
                    else:
                        nc.vector.scalar_tensor_tensor(
                            out=scr[:], in0=buf[:, i, :], scalar=1.0,
                            in1=buf[:, i, :], op0=OP.mult, op1=OP.mult,
                            accum_out=S2[:, i:i + 1])
                mean = st.tile([128, 128], FP, tag="mean")
                nc.vector.tensor_scalar(out=mean[:, 0:T], in0=S1[:, 0:T],
                                        scalar1=1.0 / C, scalar2=None, op0=OP.mult)
                msq = st.tile([128, 128], FP, tag="msq")
                nc.gpsimd.tensor_tensor(out=msq[:, 0:T], in0=mean[:, 0:T],
                                        in1=mean[:, 0:T], op=OP.mult)
                var = st.tile([128, 128], FP, tag="var")
                nc.vector.scalar_tensor_tensor(out=var[:, 0:T], in0=S2[:, 0:T],
                                               scalar=1.0 / C, in1=msq[:, 0:T],
                                               op0=OP.mult, op1=OP.subtract)
                rstd = st.tile([128, 128], FP, tag="rstd")
                nc.scalar.activation(out=rstd[:, 0:T], in_=var[:, 0:T],
                                     func=AF.Sqrt, bias=epsT[:], scale=1.0)
                nc.vector.reciprocal(out=rstd[:, 0:T], in_=rstd[:, 0:T])
                return mean, rstd

            def vec_stats(xap, P):
                stats = wk.tile([128, 6], FP, tag="bs")
                nc.vector.bn_stats(out=stats[0:P, :], in_=xap)
                mv = wk.tile([128, 2], FP, tag="mv")
                nc.vector.bn_aggr(out=mv[0:P, :], in_=stats[0:P, :])
                rstd = wk.tile([128, 1], FP, tag="rs")
                nc.scalar.activation(out=rstd[0:P, :], in_=mv[0:P, 1:2],
                                     func=AF.Sqrt, bias=epsT[0:P, :], scale=1.0)
                nc.vector.reciprocal(out=rstd[0:P, :], in_=rstd[0:P, :])
                return mv, rstd

            def transpose_sb(src_ap, dt_out, tag, k, ident=None, P=128):
                pt = ps_t.tile([128, 128], src_ap.dtype, tag="trps")
                if ident is None:
                    ident = {BF: idb, FP: idf, FR: idr}[src_ap.dtype]
                nc.tensor.transpose(pt[:, 0:P], src_ap, ident[0:P, 0:P])
                pool = wk3 if tag in ("EtT", "thatT", "aT") else wk2
                out = pool.tile([128, 128], dt_out, tag=tag)
                drain(out[:, 0:P], pt[:, 0:P], k)
                return out

            def proj_f32r(x_t_list, w_fr, n_rows, cols, kdrain, out_dt,
                          out_tag, act=None):
                """token-major x @ W, f32r. Returns sbuf tile [n_rows, cols]."""
                out = wtile([128, cols], out_dt, out_tag)
                nk = len(x_t_list)
                c0 = 0
                ci = 0
                while c0 < cols:
                    cw = min(512, cols - c0)
                    pt = ps_w.tile([128, 512], FP, tag="ptile")
                    for ki, xt in enumerate(x_t_list):
                        nc.tensor.matmul(out=pt[0:n_rows, 0:cw],
                                         lhsT=xt[:, 0:n_rows],
                                         rhs=w_fr[:, ki, c0:c0 + cw],
                                         start=(ki == 0), stop=(ki == nk - 1),
                                         skip_group_check=True)
                    if act is not None:
                        nc.scalar.activation(out=out[0:n_rows, c0:c0 + cw],
                                             in_=pt[0:n_rows, 0:cw], func=act)
                    else:
                        drain(out[0:n_rows, c0:c0 + cw], pt[0:n_rows, 0:cw],
                              kdrain + ci)
                    c0 += cw
                    ci += 1
                return out

            # =================== per-batch program ===================
            for b in range(NB):
                nc.sync.dma_start(out=Ebuf[:], in_=dram['qe'].transpose([1, 0, 2]))
                nc.sync.dma_start(out=Xb[0:M, :], in_=dram['qn'][:, :])
                geb = wtile([128, GED], FP, "xn")
                nc.sync.dma_start(out=geb[0:K, :], in_=ge_in[b, :, :])
                w1 = load_w('gmlp_w1', FP, 'B', rnd=True)
                w2 = load_w('gmlp_w2', FP, 'Cc', rnd=True)
                ge_t = [transpose_sb(geb[0:K, 128 * q:128 * (q + 1)], FR,
                                     "x_t" + str(q), q, P=K)
                        for q in range(2)]
                gh = proj_f32r(ge_t, w1, K, DH, 0, FR, "wide", act=AF.Relu)
                gh_t = [transpose_sb(gh[0:K, 128 * q:128 * (q + 1)], FR,
                                     "h_t" + str(q), q, P=K)
                        for q in range(4)]
                gx = proj_f32r(gh_t, w2, K, D, 1, FP, "px")
                nc.scalar.copy(out=Xb[M:N, :], in_=gx[0:K, :])

                for l in range(L):
                    pre = f'l{l}_'
                    T = M if l == L - 1 else N
                    wqkv = load_w(pre + 'qkv', FP, 'A', rnd=True)
                    wo = load_w(pre + 'wo', FP, 'D', rnd=True)
                    wmx1 = load_w(pre + 'mx1', FP, 'B', rnd=True)
                    wmx2 = load_w(pre + 'mx2', FP, 'Cc', rnd=True)
                    wopab = load_w(pre + 'opab', FP, 'E', rnd=True)
                    wop = load_w(pre + 'wop', FP, 'wop')
                    wb_sb = load_w(pre + 'wb', BF, 'wb')
                    w5 = load_w(pre + 'w5', BF, 'w5')
                    wto = load_w(pre + 'wto', BF, 'wto')
                    me1 = load_w(pre + 'me1', BF, 'me1')
                    me2 = load_w(pre + 'me2', BF, 'me2')

                    # ---- A: bias-site LN stats ----
                    meanA, rstdA = ln_stats(Ebuf, N)
                    mrA = st.tile([128, 128], FP, tag="mrA")
                    nc.gpsimd.tensor_tensor(out=mrA[:], in0=meanA[:], in1=rstdA[:],
                                            op=OP.mult)
                    rT = transpose_sb(rstdA[:], BF, "rT", 0)
                    mrT = transpose_sb(mrA[:], BF, "mrT", 1)

                    # ---- B: pair bias ----
                    bias_tm = big.tile([128, N * H], FP, tag="bias_tm")
                    pb = None
                    for i in range(N):
                        EtT = transpose_sb(Ebuf[:, i, :], BF, "EtT", i)
                        if i % 64 == 0:
                            pb = ps_g.tile([128, 512], FP, tag="pgrp")
                        nc.tensor.matmul(out=pb[:, (i % 64) * 8:(i % 64) * 8 + 8],
                                         lhsT=EtT[:], rhs=wb_sb[:, 0, :],
                                         start=True, stop=True,
                                         skip_group_check=True)
                        if i % 64 == 63:
                            drain(bias_tm[:, (i - 63) * 8:(i + 1) * 8], pb[:],
                                  i // 64)
                    wbcol = consts[pre + 'wb_colsum']
                    btm3 = bias_tm[:].rearrange("j (i h) -> j h i", h=H)
                    for h in range(H):
                        pbt = ps_w.tile([128, 512], FP, tag="ptile")
                        nc.tensor.transpose(pbt[:, 0:128], btm3[:, h, :], idf[:])
                        braw = wk.tile([128, 128], BF, tag="braw")
                        nc.scalar.copy(out=braw[:], in_=pbt[:, 0:128])
                        u = wk.tile([128, 128], BF, tag="biasu")
                        nc.vector.tensor_tensor(out=u[:], in0=braw[:], in1=rT[:],
                                                op=OP.mult)
                        nc.vector.scalar_tensor_tensor(
                            out=biasb[:, h, :], in0=mrT[:],
                            scalar=-float(wbcol[h]), in1=u[:],
                            op0=OP.mult, op1=OP.add)

                    # ---- C: attention ----
                    mvX, rX = vec_stats(Xb[:, :], N)
                    xn = wtile([128, D], FP, "xn")
                    sb_ts2(xn[0:N, :], Xb[:, :], mvX[0:N, 0:1], rX[0:N, :], 0)
                    xn_t = [transpose_sb(xn[0:N, 128 * q:128 * (q + 1)], FR,
                                         "x_t" + str(q), q)
                            for q in range(2)]
                    qkv = proj_f32r(xn_t, wqkv, N, 3 * D, 0, BF, "wide")
                    q_fm = [transpose_sb(qkv[0:N, 128 * q:128 * (q + 1)], BF,
                                         "q_fm" + str(q), q) for q in range(2)]
                    k_fm = [transpose_sb(qkv[0:N, 256 + 128 * q:256 + 128 * (q + 1)],
                                         BF, "k_fm" + str(q), q) for q in range(2)]
                    osb = big.tile([N, D], FR, tag="osb")
                    po = ps_g.tile([128, 512], FP, tag="pgrp")
                    for h in range(H):
                        hq, hr = h // 4, 32 * (h % 4)
                        psc = ps_w.tile([128, 512], FP, tag="ptile")
                        nc.tensor.matmul(out=psc[:, 0:128],
                                         lhsT=q_fm[hq][hr:hr + 32, :],
                                         rhs=k_fm[hq][hr:hr + 32, :],
                                         start=True, stop=True,
                                         tile_position=(hr, 0),
                                         skip_group_check=True)
                        s_sb = wk.tile([128, 128], BF, tag="s_sb")
                        nc.scalar.copy(out=s_sb[:], in_=psc[:, 0:128])
                        s2 = wk.tile([128, 128], BF, tag="s2")
                        nc.vector.tensor_tensor(out=s2[:], in0=s_sb[:],
                                                in1=biasb[:, h, :], op=OP.add)
                        nmax = wk.tile([128, 1], FP, tag="nmax")
                        nc.vector.tensor_reduce(out=nmax[:], in_=s2[:], axis=AX.X,
                                                op=OP.max, negate=True)
                        probs = wk.tile([128, 128], BF, tag="probs")
                        den = wk.tile([128, 1], FP, tag="den")
                        nc.scalar.activation(out=probs[:], in_=s2[:], func=AF.Exp,
                                             bias=nmax[:], scale=1.0,
                                             accum_out=den[:])
                        nc.vector.reciprocal(out=den[:], in_=den[:])
                        aT = transpose_sb(probs[:], BF, "aT", h)
                        nc.tensor.matmul(out=po[:, 32 * h:32 * h + 32],
                                         lhsT=aT[:],
                                         rhs=qkv[0:N, 512 + 32 * h:512 + 32 * h + 32],
                                         start=True, stop=True,
                                         skip_group_check=True)
                        nc.scalar.activation(out=osb[:, 32 * h:32 * h + 32],
                                             in_=po[:, 32 * h:32 * h + 32],
                                             func=AF.Copy, scale=den[:])
                    o_t = [transpose_sb(osb[:, 128 * q:128 * (q + 1)], FR,
                                        "x_t" + str(q), q)
                           for q in range(2)]
                    att = proj_f32r(o_t, wo, N, D, 0, FP, "px")
                    nc.vector.tensor_tensor(out=Xb[:, :], in0=att[0:N, :],
                                            in1=Xb[:, :], op=OP.add)

                    # ---- D: node mlp ----
                    mvX, rX = vec_stats(Xb[:, :], N)
                    xn2 = wtile([128, D], FP, "xn")
                    sb_ts2(xn2[0:N, :], Xb[:, :], mvX[0:N, 0:1], rX[0:N, :], 0)
                    xn2_t = [transpose_sb(xn2[0:N, 128 * q:128 * (q + 1)], FR,
                                          "x_t" + str(q), q)
                             for q in range(2)]
                    mh = proj_f32r(xn2_t, wmx1, N, DH, 0, FR, "wide", act=AF.Relu)
                    mh_t = [transpose_sb(mh[0:N, 128 * q:128 * (q + 1)], FR,
                                         "h_t" + str(q), q)
                            for q in range(4)]
                    mo = proj_f32r(mh_t, wmx2, N, D, 1, FP, "px")
                    nc.vector.tensor_tensor(out=Xb[:, :], in0=mo[0:N, :],
                                            in1=Xb[:, :], op=OP.add)

                    # ---- E: outer-product update ----
                    mvX, rX = vec_stats(Xb[:, :], N)
                    xn3 = wtile([128, D], FP, "xn")
                    sb_ts2(xn3[0:N, :], Xb[:, :], mvX[0:N, 0:1], rX[0:N, :], 0)
                    xn3_t = [transpose_sb(xn3[0:N, 128 * q:128 * (q + 1)], FR,
                                          "x_t" + str(q), q)
                             for q in range(2)]
                    oab = proj_f32r(xn3_t, wopab, N, 2 * C, 0, FP, "oab")
                    oa_t = transpose_sb(oab[0:N, 0:128], FP, "oa_t", 0)
                    ob_t = transpose_sb(oab[0:N, 128:256], BF, "ob_t", 1)
                    pop = None
                    for i in range(T):
                        wi = wk2.tile([C, C], BF, tag="wi")
                        k3 = i % 3
                        if k3 == 2:
                            nc.scalar.activation(out=wi[:], in_=wop[:, 0, :],
                                                 func=AF.Copy,
                                                 scale=oa_t[:, i:i + 1])
                        else:
                            [nc.vector, nc.gpsimd][k3].tensor_scalar(
                                out=wi[:], in0=wop[:, 0, :],
                                scalar1=oa_t[:, i:i + 1], scalar2=None,
                                op0=OP.mult)
                        if i % 4 == 0:
                            pop = ps_g.tile([128, 512], FP, tag="pgrp")
                        nc.tensor.matmul(out=pop[:, (i % 4) * 128:(i % 4 + 1) * 128],
                                         lhsT=ob_t[:], rhs=wi[:], start=True,
                                         stop=True, skip_group_check=True)
                        if i % 4 == 3:
                            psum_add_into(Ebuf[:, i - 3:i + 1, :], pop[:], i // 4)

                    # ---- F: tri projections ----
                    meanT, rstdT = ln_stats(Ebuf, T)
                    for i in range(T):
                        ethat = wk.tile([128, C], BF, tag="ethat")
                        sb_ts2(ethat[:], Ebuf[:, i, :], meanT[:, i:i + 1],
                               rstdT[:, i:i + 1], i)
                        etT = transpose_sb(ethat[:], BF, "EtT", i)
                        pA = ps_w.tile([128, 512], FP, tag="ptile")
                        nc.tensor.matmul(out=pA[:], lhsT=etT[:],
                                         rhs=w5[:, 0, 0:512], start=True,
                                         stop=True, skip_group_check=True)
                        pB = ps_w.tile([128, 512], FP, tag="ptile")
                        nc.tensor.matmul(out=pB[:, 0:128], lhsT=etT[:],
                                         rhs=w5[:, 0, 512:640], start=True,
                                         stop=True, skip_group_check=True)
                        sgm = wk.tile([128, 384], BF, tag="sgm")
                        nc.scalar.activation(out=sgm[:], in_=pA[:, 0:384],
                                             func=AF.Sigmoid)
                        rta = wk.tile([128, 128], BF, tag="rta")
                        nc.vector.tensor_copy(rta[:], pA[:, 384:512])
                        rtb = wk.tile([128, 128], BF, tag="rtb")
                        nc.vector.tensor_copy(rtb[:], pB[:, 0:128])
                        nc.vector.tensor_tensor(out=tabuf[:, i, :],
                                                in0=sgm[:, 0:128], in1=rta[:],
                                                op=OP.mult)
                        nc.gpsimd.tensor_tensor(out=tbbuf[:, i, :],
                                                in0=sgm[:, 128:256], in1=rtb[:],
                                                op=OP.mult)
                        nc.gpsimd.tensor_copy(sgbuf[:, i, :], sgm[:, 256:384])

                    # ---- G: triangular einsum (t overwrites tbbuf per channel) ----
                    pe_ = None
                    for c in range(C):
                        if c % 4 == 0:
                            pe_ = ps_g.tile([128, 512], FP, tag="pgrp")
                        nc.tensor.matmul(out=pe_[0:T, (c % 4) * 128:(c % 4) * 128 + T],
                                         lhsT=tbbuf[:, 0:T, c],
                                         rhs=tabuf[:, 0:T, c],
                                         start=True, stop=True,
                                         skip_group_check=True)
                        if c % 4 == 3:
                            src = pe_[0:T, :].rearrange("j (c i) -> j i c", c=4)
                            drain(tbbuf[0:T, 0:T, c - 3:c + 1], src[:, 0:T, :],
                                  c // 4)

                    # ---- H: ln_t + wto + gate + residual ----
                    meanL, rstdL = ln_stats(tbbuf, T, kb=1)
                    pw = None
                    for i in range(T):
                        that = wk.tile([128, C], BF, tag="that")
                        sb_ts2(that[0:T, :], tbbuf[0:T, i, :], meanL[0:T, i:i + 1],
                               rstdL[0:T, i:i + 1], i)
                        thatT = transpose_sb(that[0:T, :], BF, "thatT", i, P=T)
                        if i % 4 == 0:
                            pw = ps_g.tile([128, 512], FP, tag="pgrp")
                        nc.tensor.matmul(out=pw[0:T, (i % 4) * 128:(i % 4 + 1) * 128],
                                         lhsT=thatT[:, 0:T], rhs=wto[:, 0, :],
                                         start=True, stop=True,
                                         skip_group_check=True)
                        if i % 4 == 3:
                            i0 = i - 3
                            gtmp = wk.tile([128, 512], BF, tag="gtmp")
                            nc.scalar.copy(out=gtmp[0:T, :], in_=pw[0:T, :])
                            gt2 = wk.tile([128, 512], BF, tag="gt2")
                            sg2 = sgbuf[0:T, i0:i0 + 4, :].rearrange("p a b -> p (a b)")
                            e2 = Ebuf[0:T, i0:i0 + 4, :].rearrange("p a b -> p (a b)")
                            nc.gpsimd.tensor_tensor(
                                out=gt2[0:T, :], in0=gtmp[0:T, :],
                                in1=sg2, op=OP.mult)
                            nc.vector.tensor_tensor(
                                out=e2, in0=gt2[0:T, :], in1=e2, op=OP.add)

                    # ---- I: edge mlp ----
                    meanM, rstdM = ln_stats(Ebuf, T)
                    pm = None
                    for i in range(T):
                        ehat = wk.tile([128, C], BF, tag="ethat")
                        sb_ts2(ehat[:], Ebuf[:, i, :], meanM[:, i:i + 1],
                               rstdM[:, i:i + 1], i)
                        ehT = transpose_sb(ehat[:], BF, "EtT", i)
                        p1 = ps_w.tile([128, 512], FP, tag="ptile")
                        nc.tensor.matmul(out=p1[:, 0:256], lhsT=ehT[:],
                                         rhs=me1[:, 0, :], start=True, stop=True,
                                         skip_group_check=True)
                        hsb = wk.tile([128, CH], BF, tag="hsb")
                        nc.scalar.activation(out=hsb[:], in_=p1[:, 0:256],
                                             func=AF.Relu)
                        h_t = [transpose_sb(hsb[:, 128 * q:128 * (q + 1)], BF,
                                            "hh_t" + str(q), q + i)
                               for q in range(2)]
                        if i % 4 == 0:
                            pm = ps_g.tile([128, 512], FP, tag="pgrp")
                        for ki in range(2):
                            nc.tensor.matmul(
                                out=pm[:, (i % 4) * 128:(i % 4 + 1) * 128],
                                lhsT=h_t[ki][:], rhs=me2[:, ki, :],
                                start=(ki == 0), stop=(ki == 1),
                                skip_group_check=True)
                        if i % 4 == 3:
                            psum_add_into(Ebuf[:, i - 3:i + 1, :], pm[:], i // 4)

                # =================== heads ===================
                Xq_t = [transpose_sb(Xb[0:M, 128 * q:128 * (q + 1)], FR,
                                     "x_t" + str(q), q, P=M)
                        for q in range(2)]
                hw1 = load_w('hne_w1', FP, 'B', rnd=True)
                hw2 = load_w('hne_w2', FP, 'Cc', rnd=True)
                neh = proj_f32r(Xq_t, hw1, M, DH, 0, FR, "wide", act=AF.Relu)
                neh_t = [transpose_sb(neh[0:M, 128 * q:128 * (q + 1)], FR,
                                      "h_t" + str(q), q, P=M)
                         for q in range(4)]
                ne_sb = proj_f32r(neh_t, hw2, M, D, 1, FP, "px")
                nc.sync.dma_start(out=o_ne[b, :, :], in_=ne_sb[0:M, :])

                fw1 = load_w('hnf_w1', FP, 'B', rnd=True)
                fw2 = load_w('hnf_w2', FP, 'Cc', rnd=True)
                nfh = proj_f32r(Xq_t, fw1, M, DH, 0, FR, "wide", act=AF.Relu)
                nfh_t = [transpose_sb(nfh[0:M, 128 * q:128 * (q + 1)], FR,
                                      "h_t" + str(q), q, P=M)
                         for q in range(4)]
                nf_sb = proj_f32r(nfh_t, fw2, M, NF, 1, FP, "pxs")
                nc.sync.dma_start(out=o_nf[b, :, :], in_=nf_sb[0:M, 0:NF])

                hhnl = load_w('hhnl', FP, 'E', rnd=True)
                hnl_sb = proj_f32r(Xq_t, hhnl, M, 2 + NL, 0, FP, "pxs")
                nc.sync.dma_start(out=o_hnl[b, :, :], in_=hnl_sb[0:M, 0:1 + NL])

                haef = load_w('haef', BF, 'w5')
                hef2 = load_w('hef2', BF, 'me2')
                Ael = big.tile([M, M, 9], FP, tag="Ael")
                ef_sb = big.tile([M, M, EF], BF, tag="ef_sb")
                pef = None
                for i in range(M):
                    EtT = transpose_sb(Ebuf[:, i, :], BF, "EtT", i)
                    pae = ps_w.tile([128, 512], FP, tag="ptile")
                    nc.tensor.matmul(out=pae[0:M, 0:265], lhsT=EtT[:, 0:M],
                                     rhs=haef[:, 0, :], start=True, stop=True,
                                     skip_group_check=True)
                    drain(Ael[:, i, :], pae[0:M, 0:9], i)
                    efh = wk.tile([128, CH], BF, tag="hsb")
                    nc.scalar.activation(out=efh[0:M, :], in_=pae[0:M, 9:265],
                                         func=AF.Relu)
                    efh_t = [transpose_sb(efh[0:M, 128 * q:128 * (q + 1)], BF,
                                          "hh_t" + str(q), q + i, P=M)
                             for q in range(2)]
                    if i % 8 == 0:
                        pef = ps_g.tile([128, 512], FP, tag="pgrp")
                    for ki in range(2):
                        nc.tensor.matmul(out=pef[0:M, (i % 8) * 8:(i % 8) * 8 + 8],
                                         lhsT=efh_t[ki][:, 0:M],
                                         rhs=hef2[:, ki, :],
                                         start=(ki == 0), stop=(ki == 1),
                                         skip_group_check=True)
                    if i % 8 == 7:
                        drain(ef_sb[:, i - 7:i + 1, :],
                              pef[0:M, 0:64].rearrange("p (i e) -> p i e", e=8), i // 8)
                A_sb = big.tile([M, M], FP, tag="A_sb")
                nc.vector.tensor_copy(A_sb[:], Ael[:, :, 0])
                el_sb = big.tile([M, M, EL], BF, tag="el_sb")
                nc.scalar.copy(out=el_sb[:], in_=Ael[:, :, 1:9])
                nc.sync.dma_start(out=o_A[b, :, :], in_=A_sb[:])
                nc.sync.dma_start(out=o_el[b, :, :, :], in_=el_sb[:])
                nc.sync.dma_start(out=o_ef[b, :, :, :], in_=ef_sb[:])

    nc.compile()
    return nc


_CACHE = {}


def kernel(graph_embedding, params):
    ge = np.asarray(graph_embedding, dtype=np.float32)
    weights, consts = _prep_weights(params)

    if "nc" not in _CACHE:
        _CACHE["nc"] = _build(weights, consts)
    nc = _CACHE["nc"]

    wmaps = {k: np.asarray(v) for k, v in weights.items()}
    in_maps = []
    for core in range(NCORES):
        m = dict(wmaps)
        m["ge"] = np.ascontiguousarray(ge[core * NB:(core + 1) * NB])
        in_maps.append(m)

    rr = run_bass_kernel_spmd(nc, in_maps, list(range(NCORES)))
    cat = lambda k: np.concatenate([r[k] for r in rr.results], axis=0)
    ne = cat("o_ne")
    hnl = cat("o_hnl")
    nf = cat("o_nf")
    A = cat("o_A")
    el = cat("o_el").astype(np.float32)
    ef = cat("o_ef").astype(np.float32)
    h = np.ascontiguousarray(hnl[:, :, 0])
    nl = np.ascontiguousarray(hnl[:, :, 1:])
    A = np.ascontiguousarray(A.transpose(0, 2, 1))
    el = np.ascontiguousarray(el.transpose(0, 2, 1, 3))
    ef = np.ascontiguousarray(ef.transpose(0, 2, 1, 3))
    return (ne, h, nl, nf, A, el, ef)
